# revision 27
# baseline (speedup 1.0000x reference)
"""Trainium2 Bass kernel for nn_BASE_49821620633700 (sparse_attention).

v3: output-CHANNEL sharded across the 8 NeuronCores, zero collectives.

The final InstanceNorm normalizes each output channel over all 1024
positions, so sharding the 512 down-conv output channels 8 ways keeps the
stats fully core-local.  Per core (64 output channels):

  * gaussian path and the down conv shard 8x.  Both h-parity halves of a
    core's channels are packed into one 128-partition tile (partitions
    0-63 = h=0, 64-127 = h=1) so every matmul runs with full partitions.
  * patch attention is replicated (every core needs the full attention
    output for its down-conv contraction): 128-query score blocks (F=192
    band windows), exp straight off the score PSUM, one fused DVE op for
    the 0/1 band mask multiply + row-sum (+ host-precomputed zero-pad
    corr terms), softmax 1/Z folded into the bf16 weight cast before the
    PE transpose, value matmuls against a 9-tile position-major grid of
    x^T shared with the gaussian path, and direct SBUF->SBUF repack DMAs.
  * startup: all small constants ship as ONE f32 blob (each dma_start
    blocks its queue ~0.6us regardless of size), mAB/w2h ship as single
    partition-rearranged DMAs, and the SE pooling splits across DVE and
    ACT so the sigma tiles come up as early as possible.
  * down-conv matmuls are interleaved into the value loop (each (h,tt)
    pair fires as soon as its two repack DMAs land) so PE never drains.
  * normalize+LeakyReLU is one fused Prelu activation; a dummy Sqrt after
    the last exp prewarms the ACT table off the critical tail.

Host gathers the 8 per-core [64, 1024] outputs into the full (512, 1024).
"""
import sys

if "/opt/trn_rl_repo" not in sys.path:
    sys.path.insert(0, "/opt/trn_rl_repo")

import numpy as np
import concourse.bass as bass
import concourse.mybir as mybir
from concourse import tile
from concourse.bass_utils import run_bass_kernel_spmd

F32 = mybir.dt.float32
BF16 = mybir.dt.bfloat16
FP8 = mybir.dt.float8e4
AF = mybir.ActivationFunctionType
ALU = mybir.AluOpType

H = W = 32
HW = H * W          # 1024 positions
C = 512             # channels
R_SE = C // 16      # 32
EPS = 1e-5
KC = C // 128       # 4 channel chunks of 128
NT = 8              # 8 query tiles of 128
NG = 9              # 9 overlapping position-grid tiles of 128
NCORES = 8
OCS = C // NCORES   # 64 output channels per core

# const blob layout (f32, [128, NBLOB]): col ranges
_SW2 = 0            # rows 0:32, cols 0:512
_B2 = 0             # row 64, cols 0:512
_SW1 = 512          # cols 512:640 (4 chunks of 32)
_MASK = 640         # cols 640:832
_CORR = 832         # cols 832:840
_B1 = 840           # rows 0:32, col 840
_B2C = 841          # cols 841:845
_EPS = 845          # col 845
_ONES = 846         # row 0, cols 846:974
_FOLD = 974         # cols 974:1102: I + swap64 (cross-parity stats fold)
NBLOB = 1102


def gussin_np(v=1.5, n=32):
    d = (np.arange(n)[:, None] - np.arange(n)[None, :]).astype(np.float64) ** 2
    g = np.exp(-(d[:, None, :, None] + d[None, :, None, :]) / (2.0 * v * v)) / (
        2.0 * np.pi * v * v
    )
    g = g.reshape(n * n, n, n)
    return (g / g.sum((-1, -2), keepdims=True)).astype(np.float32)


def _bf16(a):
    import ml_dtypes

    return np.ascontiguousarray(a).astype(ml_dtypes.bfloat16)


def _fp8(a):
    return np.ascontiguousarray(a).astype(mybir.dt.np(mybir.dt.float8e4))


def prep_inputs(x, se_w1, se_b1, se_w2, se_b2, down_w):
    x = np.asarray(x, np.float32)
    xn = np.ascontiguousarray(x.reshape(C, HW))                       # (512, 1024)
    rdpad = np.zeros((HW + 160, C), np.float32)
    rdpad[64:64 + HW] = xn.T
    gus = gussin_np(1.5, H).reshape(HW, HW)
    w1 = np.asarray(down_w, np.float32)[:, :C]
    w2 = np.asarray(down_w, np.float32)[:, C:]
    m0 = w1 @ gus[0::2]                                               # (512 oc, 1024 q)
    m1 = w1 @ gus[1::2]

    blob = np.zeros((128, NBLOB), np.float32)
    blob[0:R_SE, _SW2:_SW2 + C] = np.asarray(se_w2, np.float32).T
    blob[64, _B2:_B2 + C] = np.asarray(se_b2, np.float32)
    # 1/HW pooling mean folded into the first SE matmul
    blob[:, _SW1:_SW1 + 128] = (
        np.asarray(se_w1, np.float32).T / HW
    ).reshape(KC, 128, R_SE).transpose(1, 0, 2).reshape(128, 128)
    # 0/1 band mask (128, 192)
    rho = np.arange(4)[:, None, None, None]
    cq = np.arange(W)[None, :, None, None]
    om = np.arange(6)[None, None, :, None]
    cp = np.arange(W)[None, None, None, :]
    sel = (om >= rho) & (om <= rho + 2) & (np.abs(cp - cq) <= 1)
    blob[:, _MASK:_MASK + 192] = sel.reshape(128, 192)
    # corr[q] = 3*(3 - n_valid_dx): zero-pad exp(0)=1 denominator terms
    nvdx = 1 + (np.arange(W) > 0) + (np.arange(W) < W - 1)
    corr = np.tile(3.0 * (3 - nvdx), H).astype(np.float32)
    blob[:, _CORR:_CORR + NT] = corr.reshape(NT, 128).T
    blob[0:R_SE, _B1] = np.asarray(se_b1, np.float32)
    blob[:, _B2C:_B2C + KC] = np.asarray(se_b2, np.float32).reshape(KC, 128).T
    blob[:, _EPS] = EPS
    blob[0, _ONES:_ONES + 128] = 1.0
    fold = np.eye(128, dtype=np.float32)
    fold += np.roll(fold, 64, axis=1)
    blob[:, _FOLD:_FOLD + 128] = fold

    # query permutation to parity-major pair order
    par, r_, ch = np.meshgrid(np.arange(2), np.arange(4), np.arange(16),
                              indexing="ij")
    old = (32 * r_ + 2 * ch + par).reshape(-1)
    new = (64 * par + 16 * r_ + ch).reshape(-1)
    perm = np.zeros((128, 128), np.float32)
    perm[old, new] = 1.0

    shared = {
        "xn": _bf16(xn),
        "rdpad": _bf16(rdpad),
        "blob": blob,
        "perm": _bf16(perm),
    }
    in_maps = []
    for r in range(NCORES):
        Or = slice(OCS * r, OCS * r + OCS)
        # mAB: 9 grid-aligned q-chunks; chunk j rows = positions 128j-32..128j+96
        mAB = np.zeros((NG * 128, 128), np.float32)
        for j in range(NG):
            qpos = np.arange(128 * j - 32, 128 * j + 96)
            v = (qpos >= 0) & (qpos < HW)
            mAB[128 * j + np.nonzero(v)[0], 0:OCS] = m0[Or].T[qpos[v]]
            mAB[128 * j + np.nonzero(v)[0], OCS:128] = m1[Or].T[qpos[v]]
        w2c = w2[Or].T                                                 # (512 u, 64 oc)
        w2h0 = np.concatenate([w2c, np.zeros_like(w2c)], 1)            # (512, 128)
        w2h1 = np.concatenate([np.zeros_like(w2c), w2c], 1)
        m = dict(shared)
        m["mAB"] = _bf16(
            mAB.reshape(NG, 128, 128).transpose(1, 0, 2).reshape(128, NG * 128)
        )
        m["w2h0"] = _bf16(
            w2h0.reshape(4, 128, 128).transpose(1, 0, 2).reshape(128, 4 * 128)
        )
        m["w2h1"] = _bf16(
            w2h1.reshape(4, 128, 128).transpose(1, 0, 2).reshape(128, 4 * 128)
        )
        in_maps.append(m)
    return in_maps


def build_nc():
    nc = bass.Bass(target_bir_lowering=False, debug=False)

    xn_d = nc.declare_dram_parameter("xn", [C, HW], BF16, isOutput=False)
    rdpad_d = nc.declare_dram_parameter("rdpad", [HW + 160, C], BF16, isOutput=False)
    mAB_d = nc.declare_dram_parameter("mAB", [128, NG * 128], BF16, isOutput=False)
    w2h0_d = nc.declare_dram_parameter("w2h0", [128, 4 * 128], BF16, isOutput=False)
    w2h1_d = nc.declare_dram_parameter("w2h1", [128, 4 * 128], BF16, isOutput=False)
    blob_d = nc.declare_dram_parameter("blob", [128, NBLOB], F32, isOutput=False)
    perm_d = nc.declare_dram_parameter("perm", [128, 128], BF16, isOutput=False)
    out_d = nc.declare_dram_parameter("out", [OCS, HW], F32, isOutput=True)

    with tile.TileContext(nc) as tc:
        with (
            tc.tile_pool(name="big", bufs=1) as bigp,
            tc.tile_pool(name="work", bufs=3) as workp,
        ):
            # ---------- input loads (few, fat DMAs; 3 queues) ----------
            blob = bigp.tile([128, NBLOB], F32, tag="blob", name="blob")
            nc.gpsimd.dma_start(out=blob[:], in_=blob_d[:])
            grid = []
            for j in range(NG):
                t_ = bigp.tile([128, C], BF16, tag=f"g{j}", name=f"g{j}")
                grid.append(t_)
            mab = bigp.tile([128, NG * 128], BF16, tag="mab", name="mab")
            nc.sync.dma_start(out=mab[:], in_=mAB_d[:])
            xn_sb = [
                bigp.tile([128, HW], BF16, tag=f"xn{k}", name=f"xn{k}")
                for k in range(KC)
            ]
            nc.gpsimd.dma_start(out=xn_sb[0][:], in_=xn_d[0:128, :])
            nc.gpsimd.dma_start(out=xn_sb[1][:], in_=xn_d[128:256, :])
            for j in range(NG):
                nc.sync.dma_start(
                    out=grid[j][:], in_=rdpad_d[32 + 128 * j:160 + 128 * j, :]
                )
            nc.gpsimd.dma_start(out=xn_sb[2][:], in_=xn_d[256:384, :])
            nc.gpsimd.dma_start(out=xn_sb[3][:], in_=xn_d[384:512, :])
            w2h = {}
            for hh, wd in ((0, w2h0_d), (1, w2h1_d)):
                t_ = bigp.tile([128, 4 * 128], BF16, tag=f"w2_{hh}", name=f"w2_{hh}")
                nc.sync.dma_start(out=t_[:], in_=wd[:])
                w2h[hh] = t_
            perm_sb = bigp.tile([128, 128], BF16, tag="perm", name="perm_sb")
            nc.sync.dma_start(out=perm_sb[:], in_=perm_d[:])
            # prewarm the sigmoid act table in the DMA shadow
            sgwarm = workp.tile([1, 1], F32, tag="sgwarm", bufs=1, name="sgwarm")
            nc.scalar.activation(sgwarm[:], blob[0:1, _EPS:_EPS + 1], AF.Sigmoid)

            # const views into the blob
            sw2 = blob[0:R_SE, _SW2:_SW2 + C]
            b2 = blob[64:65, _B2:_B2 + C]
            mask_v = blob[:, _MASK:_MASK + 192]
            b1 = blob[0:R_SE, _B1:_B1 + 1]
            b2c = blob[:, _B2C:_B2C + KC]
            eps_v = blob[:, _EPS:_EPS + 1]
            ones_row = blob[0:1, _ONES:_ONES + 128]

            # sigma channel-pair tiles for DoubleRow scores: tile m holds
            # channels 256m+128j+p at flat col 1088j+32+q (fp8, guard cols)
            scm_sb = [
                bigp.tile([128, 2 * (HW + 64)], FP8, tag=f"scm{k}", name=f"scm{k}")
                for k in range(2)
            ]
            f_sb = {}
            for hh in range(2):
                for tt in range(4):
                    f_sb[(hh, tt)] = bigp.tile(
                        [128, C], BF16, tag=f"f{hh}_{tt}", name=f"f{hh}_{tt}"
                    )
            oa_sb = bigp.tile([128, C], F32, tag="oa_sb", name="oa_sb")
            ybc_sb = bigp.tile([128, C], F32, tag="ybc_sb", name="ybc_sb")
            o_sb = bigp.tile([128, C], F32, tag="o_sb", name="o_sb")
            e_bf = {
                t: bigp.tile([128, 192], BF16, tag=f"ebf{t}", name=f"ebf{t}")
                for t in range(NT)
            }

            # ---------- gaussian path + SE chain (scoped PSUM) ----------
            with tc.tile_pool(name="ps_se", bufs=1, space="PSUM") as pse:
                # SE pooling: half-reduces split across DVE and ACT
                ysum8 = workp.tile([128, 2 * KC], F32, tag="ysum8", bufs=1,
                                   name="ysum8")
                pjunk = workp.tile([128, 512], F32, tag="pjunk", bufs=1,
                                   name="pjunk")
                for hk in range(2 * KC):
                    k, hf = hk // 2, hk % 2
                    if hf == 0:
                        nc.vector.reduce_sum(
                            ysum8[:, hk:hk + 1],
                            xn_sb[k][:, 0:512],
                            axis=mybir.AxisListType.X,
                        )
                    else:
                        nc.scalar.activation(
                            pjunk[:], xn_sb[k][:, 512:1024],
                            AF.Copy, accum_out=ysum8[:, hk:hk + 1],
                        )
                y1_ps = pse.tile([R_SE, 1], F32, tag="y1", name="y1_ps")
                for hk in range(2 * KC):
                    k = hk // 2
                    nc.tensor.matmul(
                        y1_ps[:], blob[:, _SW1 + 32 * k:_SW1 + 32 * (k + 1)],
                        ysum8[:, hk:hk + 1], start=(hk == 0),
                        stop=(hk == 2 * KC - 1),
                    )
                y1_sb = workp.tile([R_SE, 1], F32, tag="y1_sb", name="y1_sb")
                nc.scalar.activation(y1_sb[:], y1_ps[:], AF.Relu, bias=b1)

                y2c_ps = pse.tile([128, KC], F32, tag="y2c", name="y2c_ps")
                for k in range(KC):
                    nc.tensor.matmul(
                        y2c_ps[:, k:k + 1], sw2[:, 128 * k:128 * (k + 1)],
                        y1_sb[:], start=True, stop=True,
                    )
                y2c_sb = workp.tile([128, KC], F32, tag="y2cs", name="y2c_sb")
                for k in range(KC):
                    nc.scalar.activation(
                        y2c_sb[:, k:k + 1], y2c_ps[:, k:k + 1], AF.Sigmoid,
                        bias=b2c[:, k:k + 1],
                    )

                y2_ps = pse.tile([1, C], F32, tag="y2", name="y2_ps")
                nc.tensor.matmul(y2_ps[:], y1_sb[:], sw2, start=True, stop=True)
                y2pb = workp.tile([1, C], F32, tag="y2pb", name="y2pb")
                nc.vector.tensor_tensor(out=y2pb[:], in0=y2_ps[:], in1=b2,
                                        op=ALU.add)
                y2_sb = workp.tile([1, C], F32, tag="y2s", name="y2_sb")
                nc.scalar.activation(y2_sb[:], y2pb[:], AF.Sigmoid)

                # sigma tiles: sigmoid(y_c * x) fused scale+activation,
                # written straight into the channel-pair halves
                for k in range(KC):
                    mm, jj = k // 2, k % 2
                    nc.scalar.activation(
                        scm_sb[mm][:, 1088 * jj + 32:1088 * jj + 32 + HW],
                        xn_sb[k][:], AF.Sigmoid, scale=y2c_sb[:, k:k + 1],
                    )

                # gaussian path (PE fills while ACT runs the sigmas)
                oa_ps = pse.tile([128, C], F32, tag="oa_ps", name="oa_ps")
                for j in range(NG):
                    nc.tensor.matmul(
                        oa_ps[:], mab[:, 128 * j:128 * (j + 1)], grid[j][:],
                        start=(j == 0), stop=(j == NG - 1),
                    )
                nc.vector.tensor_copy(oa_sb[:], oa_ps[:])
                for mm in range(2):
                    for jj in range(2):
                        nc.gpsimd.memset(
                            scm_sb[mm][:, 1088 * jj:1088 * jj + 32], 0.0)
                        nc.gpsimd.memset(
                            scm_sb[mm][:, 1088 * jj + 32 + HW:1088 * (jj + 1)], 0.0)

            # ---------- attention + down conv (main PSUM) ----------
            with tc.tile_pool(name="ps_main", bufs=1, space="PSUM") as psm:
                for t in range(NT):
                    sc_ps = psm.tile([128, 192], F32, tag="sc", bufs=2,
                                     name=f"sc{t}")
                    for mm in range(2):
                        sp = scm_sb[mm][:].rearrange("p (j q) -> p j q", j=2)
                        nc.tensor.matmul(
                            sc_ps[:],
                            sp[:, :, 32 + 128 * t:160 + 128 * t],
                            sp[:, :, 128 * t:128 * t + 192],
                            start=(mm == 0), stop=(mm == 1),
                            perf_mode=mybir.MatmulPerfMode.DoubleRow,
                        )
                    e_f = workp.tile([128, 192], F32, tag="e_f", name=f"e_f{t}")
                    nc.scalar.activation(e_f[:], sc_ps[:], AF.Exp, scale=1.0 / C)
                    e_m = workp.tile([128, 192], BF16, tag="e_m", name=f"e_m{t}")
                    z1 = workp.tile([128, 1], F32, tag="z1", name=f"z1_{t}")
                    nc.vector.scalar_tensor_tensor(
                        out=e_m[:], in0=e_f[:], scalar=1.0, in1=mask_v,
                        op0=ALU.mult, op1=ALU.mult, accum_out=z1[:],
                    )
                    nc.gpsimd.tensor_tensor(
                        out=z1[:], in0=z1[:], in1=blob[:, _CORR + t:_CORR + t + 1],
                        op=ALU.add,
                    )
                    rinv = workp.tile([128, 1], F32, tag="rinv", name=f"ri{t}")
                    nc.vector.reciprocal(rinv[:], z1[:])
                    # softmax scale-cast on Pool: relieves the DVE stream that
                    # also carries the eT copies for the value matmuls
                    nc.gpsimd.tensor_scalar_mul(e_bf[t][:], e_m[:], rinv[:])
                    if t == NT - 1:
                        # dep-pinned sqrt-table prewarm: depends on the last
                        # exp via rinv(7), so it lands after all exps and the
                        # load overlaps the value loop
                        sqwarm = workp.tile([1, 1], F32, tag="sqwarm", bufs=1,
                                            name="sqwarm")
                        nc.scalar.activation(sqwarm[:], rinv[0:1, :], AF.Sqrt)
                    if t == 0:
                        o_ps = psm.tile([128, C], F32, tag="o_ps", bufs=1,
                                        name="o_ps")
                        ndc = 0
                    # value path for this block, interleaved so its DVE/PE ops
                    # don't queue behind the later blocks' softmax chains
                    eTa_ps = psm.tile([128, 128], BF16, tag="eTa", bufs=1,
                                      name=f"eTa{t}")
                    nc.tensor.transpose(eTa_ps[:], e_bf[t][:, 0:128], perm_sb[:])
                    eTb_ps = psm.tile([64, 128], BF16, tag="eTb", bufs=1,
                                      name=f"eTb{t}")
                    nc.tensor.transpose(eTb_ps[:], e_bf[t][:, 128:192], perm_sb[:])
                    eTa_sb = workp.tile([128, 128], BF16, tag="eTa_sb",
                                        name=f"eTas{t}")
                    nc.vector.tensor_copy(eTa_sb[:], eTa_ps[:])
                    eTb_sb = workp.tile([64, 128], BF16, tag="eTb_sb",
                                        name=f"eTbs{t}")
                    nc.vector.tensor_copy(eTb_sb[:], eTb_ps[:])
                    cc_ps = psm.tile([128, C], F32, tag="cc", bufs=2, name=f"cc{t}")
                    nc.tensor.matmul(cc_ps[:], eTa_sb[:], grid[t][:],
                                     start=True, stop=False)
                    nc.tensor.matmul(cc_ps[:], eTb_sb[:], grid[t + 1][0:64, :],
                                     start=False, stop=True)
                    c_bf = workp.tile([128, C], BF16, tag="c_bf", name=f"cb{t}")
                    nc.scalar.activation(c_bf[:], cc_ps[:], AF.Copy)
                    for par in range(2):
                        nc.sync.dma_start(
                            out=f_sb[(par, t // 2)][64 * (t % 2):64 * (t % 2) + 64, :],
                            in_=c_bf[64 * par:64 * par + 64, :],
                        )
                    # fire down-conv pairs as soon as their f tiles are complete
                    if t % 2 == 1:
                        tt = t // 2
                        for hh in range(2):
                            nc.tensor.matmul(
                                o_ps[:],
                                w2h[hh][:, 128 * tt:128 * (tt + 1)],
                                f_sb[(hh, tt)][:],
                                start=(ndc == 0), stop=(ndc == 7),
                            )
                            ndc += 1

                # gate broadcast row -> [128, C], reusing the cc psum ring
                ybc_ps = psm.tile([128, C], F32, tag="cc", bufs=2, name="ybc_ps")
                nc.tensor.matmul(ybc_ps[:], ones_row, y2_sb[:],
                                 start=True, stop=True)
                nc.vector.tensor_copy(ybc_sb[:], ybc_ps[:])

                # merge + gate + stats
                ab = workp.tile([128, C], F32, tag="ab", bufs=1, name="ab")
                nc.vector.tensor_tensor(
                    out=ab[:], in0=oa_sb[:], in1=o_ps[:], op=ALU.add
                )
                sums = workp.tile([128, 2], F32, tag="sums", name="sums")
                nc.vector.scalar_tensor_tensor(
                    out=o_sb[:], in0=ab[:], scalar=1.0, in1=ybc_sb[:],
                    op0=ALU.mult, op1=ALU.mult, accum_out=sums[:, 0:1],
                )
                sqjunk = workp.tile([128, C], F32, tag="sqjunk", bufs=1, name="sqjunk")
                nc.scalar.activation(
                    sqjunk[:], o_sb[:], AF.Square, accum_out=sums[:, 1:2]
                )
                # cross-parity fold on PE: comb[p] = sums[p] + sums[p^64]
                comb_ps = psm.tile([128, 2], F32, tag="comb", bufs=1,
                                   name="comb_ps")
                nc.tensor.matmul(
                    comb_ps[:], blob[:, _FOLD:_FOLD + 128], sums[:],
                    start=True, stop=True,
                )
                stats2 = workp.tile([128, 2], F32, tag="stats2", name="stats2")
                nc.vector.tensor_scalar_mul(stats2[:], comb_ps[:], 1.0 / HW)
                m2 = workp.tile([128, 1], F32, tag="m2", name="m2")
                nc.vector.tensor_tensor(
                    out=m2[:], in0=stats2[:, 0:1], in1=stats2[:, 0:1], op=ALU.mult
                )
                var = workp.tile([128, 1], F32, tag="var", name="var")
                nc.vector.tensor_tensor(
                    out=var[:], in0=stats2[:, 1:2], in1=m2[:], op=ALU.subtract
                )
                std = workp.tile([128, 1], F32, tag="std", name="std")
                nc.scalar.activation(std[:], var[:], AF.Sqrt, bias=eps_v)
                norm2 = workp.tile([128, 2], F32, tag="norm2", name="norm2")
                nc.vector.reciprocal(norm2[:, 0:1], std[:])
                nc.vector.scalar_tensor_tensor(
                    out=norm2[:, 1:2], in0=stats2[:, 0:1], scalar=-1.0,
                    in1=norm2[:, 0:1], op0=ALU.mult, op1=ALU.mult,
                )
                outt = workp.tile([128, C], F32, tag="outt", bufs=1, name="outt")
                nc.scalar.activation(
                    outt[:], o_sb[:], AF.Prelu,
                    bias=norm2[:, 1:2], scale=norm2[:, 0:1], alpha=0.2,
                )
                nc.sync.dma_start(out=out_d[:, 0:C], in_=outt[0:64, :])
                nc.sync.dma_start(out=out_d[:, C:HW], in_=outt[64:128, :])

    return nc


def _split_drain_waits(nc, keep=1):
    """This walrus build allows at most 1 sync wait per instruction; hoist the
    extras onto preceding NoOps on the same engine."""
    n = 0
    for f in nc.m.functions:
        for bb in f.blocks:
            newlist = []
            for ins in bb.instructions:
                si = getattr(ins, "sync_info", None)
                if si is not None and si.on_wait and len(si.on_wait) > keep:
                    waits = list(si.on_wait)
                    for w in waits[:-keep]:
                        nop = mybir.InstNoOp(name=f"I-dw{n}", ins=[], outs=[])
                        n += 1
                        nop.engine = ins.engine
                        nop.sync_info = mybir.SyncInfo(on_wait=[w], on_update=[])
                        newlist.append(nop)
                    si.on_wait = waits[-keep:]
                newlist.append(ins)
            bb.instructions = newlist
    return n


_BUILT = None


def get_built():
    global _BUILT
    if _BUILT is None:
        nc = build_nc()
        _split_drain_waits(nc)
        _BUILT = nc
    return _BUILT


def kernel(x, se_w1, se_b1, se_w2, se_b2, down_w, _trace=False):
    in_maps = prep_inputs(x, se_w1, se_b1, se_w2, se_b2, down_w)
    nc = get_built()
    res = run_bass_kernel_spmd(nc, in_maps, list(range(NCORES)), trace=_trace)
    full = np.concatenate(
        [np.asarray(res.results[r]["out"], np.float32) for r in range(NCORES)], 0
    ).reshape(1, C, H, W)
    if _trace:
        return full, res
    return full


# revision 28
# speedup vs baseline: 1.2753x; 1.2753x over previous
"""Trainium2 Bass kernel for nn_BASE_49821620633700 (sparse_attention).

v3: output-CHANNEL sharded across the 8 NeuronCores, zero collectives.

The final InstanceNorm normalizes each output channel over all 1024
positions, so sharding the 512 down-conv output channels 8 ways keeps the
stats fully core-local.  Per core (64 output channels):

  * gaussian path and the down conv shard 8x.  Both h-parity halves of a
    core's channels are packed into one 128-partition tile (partitions
    0-63 = h=0, 64-127 = h=1) so every matmul runs with full partitions.
  * patch attention is replicated (every core needs the full attention
    output for its down-conv contraction): 128-query score blocks (F=192
    band windows), exp straight off the score PSUM, one fused DVE op for
    the 0/1 band mask multiply + row-sum (+ host-precomputed zero-pad
    corr terms), softmax 1/Z folded into the bf16 weight cast before the
    PE transpose, value matmuls against a 9-tile position-major grid of
    x^T shared with the gaussian path, and direct SBUF->SBUF repack DMAs.
  * startup: all small constants ship as ONE f32 blob (each dma_start
    blocks its queue ~0.6us regardless of size), mAB/w2h ship as single
    partition-rearranged DMAs, and the SE pooling splits across DVE and
    ACT so the sigma tiles come up as early as possible.
  * down-conv matmuls are interleaved into the value loop (each (h,tt)
    pair fires as soon as its two repack DMAs land) so PE never drains.
  * normalize+LeakyReLU is one fused Prelu activation; a dummy Sqrt after
    the last exp prewarms the ACT table off the critical tail.

Host gathers the 8 per-core [64, 1024] outputs into the full (512, 1024).
"""
import sys

if "/opt/trn_rl_repo" not in sys.path:
    sys.path.insert(0, "/opt/trn_rl_repo")

import numpy as np
import concourse.bass as bass
import concourse.mybir as mybir
from concourse import tile
from concourse.bass_utils import run_bass_kernel_spmd

F32 = mybir.dt.float32
BF16 = mybir.dt.bfloat16
FP8 = mybir.dt.float8e4
AF = mybir.ActivationFunctionType
ALU = mybir.AluOpType

H = W = 32
HW = H * W          # 1024 positions
C = 512             # channels
R_SE = C // 16      # 32
EPS = 1e-5
KC = C // 128       # 4 channel chunks of 128
NT = 8              # 8 query tiles of 128
NG = 9              # 9 overlapping position-grid tiles of 128
NCORES = 8
OCS = C // NCORES   # 64 output channels per core

# const blob layout (f32, [128, NBLOB]): col ranges
_SW2 = 0            # rows 0:32, cols 0:512
_B2 = 0             # row 64, cols 0:512
_SW1 = 512          # cols 512:640 (4 chunks of 32)
_MASK = 640         # cols 640:832
_CORR = 832         # cols 832:840
_B1 = 840           # rows 0:32, col 840
_B2C = 841          # cols 841:845
_EPS = 845          # col 845
_ONES = 846         # row 0, cols 846:974
_FOLD = 974         # cols 974:1102: I + swap64 (cross-parity stats fold)
NBLOB = 1102


def gussin_np(v=1.5, n=32):
    d = (np.arange(n)[:, None] - np.arange(n)[None, :]).astype(np.float64) ** 2
    g = np.exp(-(d[:, None, :, None] + d[None, :, None, :]) / (2.0 * v * v)) / (
        2.0 * np.pi * v * v
    )
    g = g.reshape(n * n, n, n)
    return (g / g.sum((-1, -2), keepdims=True)).astype(np.float32)


def _bf16(a):
    import ml_dtypes

    return np.ascontiguousarray(a).astype(ml_dtypes.bfloat16)


def _fp8(a):
    return np.ascontiguousarray(a).astype(mybir.dt.np(mybir.dt.float8e4))


def prep_inputs(x, se_w1, se_b1, se_w2, se_b2, down_w):
    x = np.asarray(x, np.float32)
    xn = np.ascontiguousarray(x.reshape(C, HW))                       # (512, 1024)
    rdpad = np.zeros((HW + 160, C), np.float32)
    rdpad[64:64 + HW] = xn.T
    gus = gussin_np(1.5, H).reshape(HW, HW)
    w1 = np.asarray(down_w, np.float32)[:, :C]
    w2 = np.asarray(down_w, np.float32)[:, C:]
    m0 = w1 @ gus[0::2]                                               # (512 oc, 1024 q)
    m1 = w1 @ gus[1::2]

    blob = np.zeros((128, NBLOB), np.float32)
    blob[0:R_SE, _SW2:_SW2 + C] = np.asarray(se_w2, np.float32).T
    blob[64, _B2:_B2 + C] = np.asarray(se_b2, np.float32)
    # 1/HW pooling mean folded into the first SE matmul
    blob[:, _SW1:_SW1 + 128] = (
        np.asarray(se_w1, np.float32).T / HW
    ).reshape(KC, 128, R_SE).transpose(1, 0, 2).reshape(128, 128)
    # 0/1 band mask (128, 192)
    rho = np.arange(4)[:, None, None, None]
    cq = np.arange(W)[None, :, None, None]
    om = np.arange(6)[None, None, :, None]
    cp = np.arange(W)[None, None, None, :]
    sel = (om >= rho) & (om <= rho + 2) & (np.abs(cp - cq) <= 1)
    blob[:, _MASK:_MASK + 192] = sel.reshape(128, 192)
    # corr[q] = 3*(3 - n_valid_dx): zero-pad exp(0)=1 denominator terms
    nvdx = 1 + (np.arange(W) > 0) + (np.arange(W) < W - 1)
    corr = np.tile(3.0 * (3 - nvdx), H).astype(np.float32)
    blob[:, _CORR:_CORR + NT] = corr.reshape(NT, 128).T
    blob[0:R_SE, _B1] = np.asarray(se_b1, np.float32)
    blob[:, _B2C:_B2C + KC] = np.asarray(se_b2, np.float32).reshape(KC, 128).T
    blob[:, _EPS] = EPS
    blob[0, _ONES:_ONES + 128] = 1.0
    fold = np.eye(128, dtype=np.float32)
    fold += np.roll(fold, 64, axis=1)
    blob[:, _FOLD:_FOLD + 128] = fold

    # query permutation to parity-major pair order
    par, r_, ch = np.meshgrid(np.arange(2), np.arange(4), np.arange(16),
                              indexing="ij")
    old = (32 * r_ + 2 * ch + par).reshape(-1)
    new = (64 * par + 16 * r_ + ch).reshape(-1)
    perm = np.zeros((128, 128), np.float32)
    perm[old, new] = 1.0

    shared = {
        "xn": _bf16(xn),
        "rdpad": _bf16(rdpad),
        "blob": blob,
        "perm": _bf16(perm),
    }
    in_maps = []
    for r in range(NCORES):
        Or = slice(OCS * r, OCS * r + OCS)
        # mAB: 9 grid-aligned q-chunks; chunk j rows = positions 128j-32..128j+96
        mAB = np.zeros((NG * 128, 128), np.float32)
        for j in range(NG):
            qpos = np.arange(128 * j - 32, 128 * j + 96)
            v = (qpos >= 0) & (qpos < HW)
            mAB[128 * j + np.nonzero(v)[0], 0:OCS] = m0[Or].T[qpos[v]]
            mAB[128 * j + np.nonzero(v)[0], OCS:128] = m1[Or].T[qpos[v]]
        w2c = w2[Or].T                                                 # (512 u, 64 oc)
        w2h0 = np.concatenate([w2c, np.zeros_like(w2c)], 1)            # (512, 128)
        w2h1 = np.concatenate([np.zeros_like(w2c), w2c], 1)
        m = dict(shared)
        m["mAB"] = _bf16(
            mAB.reshape(NG, 128, 128).transpose(1, 0, 2).reshape(128, NG * 128)
        )
        m["w2h0"] = _bf16(
            w2h0.reshape(4, 128, 128).transpose(1, 0, 2).reshape(128, 4 * 128)
        )
        m["w2h1"] = _bf16(
            w2h1.reshape(4, 128, 128).transpose(1, 0, 2).reshape(128, 4 * 128)
        )
        in_maps.append(m)
    return in_maps


def build_nc():
    nc = bass.Bass(target_bir_lowering=False, debug=False)

    xn_d = nc.declare_dram_parameter("xn", [C, HW], BF16, isOutput=False)
    rdpad_d = nc.declare_dram_parameter("rdpad", [HW + 160, C], BF16, isOutput=False)
    mAB_d = nc.declare_dram_parameter("mAB", [128, NG * 128], BF16, isOutput=False)
    w2h0_d = nc.declare_dram_parameter("w2h0", [128, 4 * 128], BF16, isOutput=False)
    w2h1_d = nc.declare_dram_parameter("w2h1", [128, 4 * 128], BF16, isOutput=False)
    blob_d = nc.declare_dram_parameter("blob", [128, NBLOB], F32, isOutput=False)
    perm_d = nc.declare_dram_parameter("perm", [128, 128], BF16, isOutput=False)
    out_d = nc.declare_dram_parameter("out", [OCS, HW], F32, isOutput=True)

    with tile.TileContext(nc) as tc:
        with (
            tc.tile_pool(name="big", bufs=1) as bigp,
            tc.tile_pool(name="work", bufs=3) as workp,
        ):
            # ---------- input loads (few, fat DMAs; 3 queues) ----------
            blob = bigp.tile([128, NBLOB], F32, tag="blob", name="blob")
            nc.gpsimd.dma_start(out=blob[:], in_=blob_d[:])
            grid = []
            for j in range(NG):
                t_ = bigp.tile([128, C], BF16, tag=f"g{j}", name=f"g{j}")
                grid.append(t_)
            mab = bigp.tile([128, NG * 128], BF16, tag="mab", name="mab")
            nc.sync.dma_start(out=mab[:], in_=mAB_d[:])
            xn_sb = [
                bigp.tile([128, HW], BF16, tag=f"xn{k}", name=f"xn{k}")
                for k in range(KC)
            ]
            nc.gpsimd.dma_start(out=xn_sb[0][:], in_=xn_d[0:128, :])
            nc.gpsimd.dma_start(out=xn_sb[1][:], in_=xn_d[128:256, :])
            for j in range(NG):
                nc.sync.dma_start(
                    out=grid[j][:], in_=rdpad_d[32 + 128 * j:160 + 128 * j, :]
                )
            nc.gpsimd.dma_start(out=xn_sb[2][:], in_=xn_d[256:384, :])
            nc.gpsimd.dma_start(out=xn_sb[3][:], in_=xn_d[384:512, :])
            w2h = {}
            for hh, wd in ((0, w2h0_d), (1, w2h1_d)):
                t_ = bigp.tile([128, 4 * 128], BF16, tag=f"w2_{hh}", name=f"w2_{hh}")
                nc.sync.dma_start(out=t_[:], in_=wd[:])
                w2h[hh] = t_
            perm_sb = bigp.tile([128, 128], BF16, tag="perm", name="perm_sb")
            nc.sync.dma_start(out=perm_sb[:], in_=perm_d[:])
            # prewarm the sigmoid act table in the DMA shadow
            sgwarm = workp.tile([1, 1], F32, tag="sgwarm", bufs=1, name="sgwarm")
            nc.scalar.activation(sgwarm[:], blob[0:1, _EPS:_EPS + 1], AF.Sigmoid)

            # const views into the blob
            sw2 = blob[0:R_SE, _SW2:_SW2 + C]
            b2 = blob[64:65, _B2:_B2 + C]
            mask_v = blob[:, _MASK:_MASK + 192]
            b1 = blob[0:R_SE, _B1:_B1 + 1]
            b2c = blob[:, _B2C:_B2C + KC]
            eps_v = blob[:, _EPS:_EPS + 1]
            ones_row = blob[0:1, _ONES:_ONES + 128]

            # sigma channel-pair tiles for DoubleRow scores: tile m holds
            # channels 256m+128j+p at flat col 1088j+32+q (fp8, guard cols)
            scm_sb = [
                bigp.tile([128, 2 * (HW + 64)], FP8, tag=f"scm{k}", name=f"scm{k}")
                for k in range(2)
            ]
            f_sb = {}
            for hh in range(2):
                for tt in range(4):
                    f_sb[(hh, tt)] = bigp.tile(
                        [128, C], BF16, tag=f"f{hh}_{tt}", name=f"f{hh}_{tt}"
                    )
            oa_sb = bigp.tile([128, C], F32, tag="oa_sb", name="oa_sb")
            ybc_sb = bigp.tile([128, C], F32, tag="ybc_sb", name="ybc_sb")
            o_sb = bigp.tile([128, C], F32, tag="o_sb", name="o_sb")
            e_bf = {
                t: bigp.tile([128, 192], BF16, tag=f"ebf{t}", name=f"ebf{t}")
                for t in range(NT)
            }

            # ---------- gaussian path + SE chain (scoped PSUM) ----------
            with tc.tile_pool(name="ps_se", bufs=1, space="PSUM") as pse:
                # SE pooling: half-reduces split across DVE and ACT
                ysum8 = workp.tile([128, 2 * KC], F32, tag="ysum8", bufs=1,
                                   name="ysum8")
                pjunk = workp.tile([128, 512], F32, tag="pjunk", bufs=1,
                                   name="pjunk")
                for hk in range(2 * KC):
                    k, hf = hk // 2, hk % 2
                    if hf == 0:
                        nc.vector.reduce_sum(
                            ysum8[:, hk:hk + 1],
                            xn_sb[k][:, 0:512],
                            axis=mybir.AxisListType.X,
                        )
                    else:
                        nc.scalar.activation(
                            pjunk[:], xn_sb[k][:, 512:1024],
                            AF.Copy, accum_out=ysum8[:, hk:hk + 1],
                        )
                y1_ps = pse.tile([R_SE, 1], F32, tag="y1", name="y1_ps")
                for hk in range(2 * KC):
                    k = hk // 2
                    nc.tensor.matmul(
                        y1_ps[:], blob[:, _SW1 + 32 * k:_SW1 + 32 * (k + 1)],
                        ysum8[:, hk:hk + 1], start=(hk == 0),
                        stop=(hk == 2 * KC - 1),
                    )
                y1_sb = workp.tile([R_SE, 1], F32, tag="y1_sb", name="y1_sb")
                nc.scalar.activation(y1_sb[:], y1_ps[:], AF.Relu, bias=b1)

                y2c_ps = pse.tile([128, KC], F32, tag="y2c", name="y2c_ps")
                for k in range(KC):
                    nc.tensor.matmul(
                        y2c_ps[:, k:k + 1], sw2[:, 128 * k:128 * (k + 1)],
                        y1_sb[:], start=True, stop=True,
                    )
                y2c_sb = workp.tile([128, KC], F32, tag="y2cs", name="y2c_sb")
                for k in range(KC):
                    nc.scalar.activation(
                        y2c_sb[:, k:k + 1], y2c_ps[:, k:k + 1], AF.Sigmoid,
                        bias=b2c[:, k:k + 1],
                    )

                y2_ps = pse.tile([1, C], F32, tag="y2", name="y2_ps")
                nc.tensor.matmul(y2_ps[:], y1_sb[:], sw2, start=True, stop=True)
                y2pb = workp.tile([1, C], F32, tag="y2pb", name="y2pb")
                nc.vector.tensor_tensor(out=y2pb[:], in0=y2_ps[:], in1=b2,
                                        op=ALU.add)
                y2_sb = workp.tile([1, C], F32, tag="y2s", name="y2_sb")
                nc.scalar.activation(y2_sb[:], y2pb[:], AF.Sigmoid)

                # sigma tiles: sigmoid(y_c * x) fused scale+activation,
                # written straight into the channel-pair halves
                for k in range(KC):
                    mm, jj = k // 2, k % 2
                    nc.scalar.activation(
                        scm_sb[mm][:, 1088 * jj + 32:1088 * jj + 32 + HW],
                        xn_sb[k][:], AF.Sigmoid, scale=y2c_sb[:, k:k + 1],
                    )

                # gaussian path (PE fills while ACT runs the sigmas)
                oa_ps = pse.tile([128, C], F32, tag="oa_ps", name="oa_ps")
                for j in range(NG):
                    nc.tensor.matmul(
                        oa_ps[:], mab[:, 128 * j:128 * (j + 1)], grid[j][:],
                        start=(j == 0), stop=(j == NG - 1),
                    )
                nc.vector.tensor_copy(oa_sb[:], oa_ps[:])
                for mm in range(2):
                    for jj in range(2):
                        nc.gpsimd.memset(
                            scm_sb[mm][:, 1088 * jj:1088 * jj + 32], 0.0)
                        nc.gpsimd.memset(
                            scm_sb[mm][:, 1088 * jj + 32 + HW:1088 * (jj + 1)], 0.0)

            # ---------- attention + down conv (main PSUM) ----------
            with tc.tile_pool(name="ps_main", bufs=1, space="PSUM") as psm:
                for t in range(NT):
                    sc_ps = psm.tile([128, 192], F32, tag="sc", bufs=2,
                                     name=f"sc{t}")
                    for mm in range(2):
                        sp = scm_sb[mm][:].rearrange("p (j q) -> p j q", j=2)
                        nc.tensor.matmul(
                            sc_ps[:],
                            sp[:, :, 32 + 128 * t:160 + 128 * t],
                            sp[:, :, 128 * t:128 * t + 192],
                            start=(mm == 0), stop=(mm == 1),
                            perf_mode=mybir.MatmulPerfMode.DoubleRow,
                        )
                    e_f = workp.tile([128, 192], F32, tag="e_f", name=f"e_f{t}")
                    nc.scalar.activation(e_f[:], sc_ps[:], AF.Exp, scale=1.0 / C)
                    e_m = workp.tile([128, 192], BF16, tag="e_m", name=f"e_m{t}")
                    z1 = workp.tile([128, 1], F32, tag="z1", name=f"z1_{t}")
                    nc.vector.scalar_tensor_tensor(
                        out=e_m[:], in0=e_f[:], scalar=1.0, in1=mask_v,
                        op0=ALU.mult, op1=ALU.mult, accum_out=z1[:],
                    )
                    nc.vector.tensor_tensor(
                        out=z1[:], in0=z1[:], in1=blob[:, _CORR + t:_CORR + t + 1],
                        op=ALU.add,
                    )
                    rinv = workp.tile([128, 1], F32, tag="rinv", name=f"ri{t}")
                    nc.vector.reciprocal(rinv[:], z1[:])
                    nc.vector.tensor_scalar_mul(e_bf[t][:], e_m[:], rinv[:])
                    if t == NT - 1:
                        # dep-pinned sqrt-table prewarm: depends on the last
                        # exp via rinv(7), so it lands after all exps and the
                        # load overlaps the value loop
                        sqwarm = workp.tile([1, 1], F32, tag="sqwarm", bufs=1,
                                            name="sqwarm")
                        nc.scalar.activation(sqwarm[:], rinv[0:1, :], AF.Sqrt)
                    if t == 0:
                        o_ps = psm.tile([128, C], F32, tag="o_ps", bufs=1,
                                        name="o_ps")
                        ndc = 0
                    # value path for this block, interleaved so its DVE/PE ops
                    # don't queue behind the later blocks' softmax chains
                    eTa_ps = psm.tile([128, 128], BF16, tag="eTa", bufs=1,
                                      name=f"eTa{t}")
                    nc.tensor.transpose(eTa_ps[:], e_bf[t][:, 0:128], perm_sb[:])
                    eTb_ps = psm.tile([64, 128], BF16, tag="eTb", bufs=1,
                                      name=f"eTb{t}")
                    nc.tensor.transpose(eTb_ps[:], e_bf[t][:, 128:192], perm_sb[:])
                    eTa_sb = workp.tile([128, 128], BF16, tag="eTa_sb",
                                        name=f"eTas{t}")
                    nc.vector.tensor_copy(eTa_sb[:], eTa_ps[:])
                    eTb_sb = workp.tile([64, 128], BF16, tag="eTb_sb",
                                        name=f"eTbs{t}")
                    nc.vector.tensor_copy(eTb_sb[:], eTb_ps[:])
                    cc_ps = psm.tile([128, C], F32, tag="cc", bufs=2, name=f"cc{t}")
                    nc.tensor.matmul(cc_ps[:], eTa_sb[:], grid[t][:],
                                     start=True, stop=False)
                    nc.tensor.matmul(cc_ps[:], eTb_sb[:], grid[t + 1][0:64, :],
                                     start=False, stop=True)
                    c_bf = workp.tile([128, C], BF16, tag="c_bf", name=f"cb{t}")
                    nc.scalar.activation(c_bf[:], cc_ps[:], AF.Copy)
                    for par in range(2):
                        nc.sync.dma_start(
                            out=f_sb[(par, t // 2)][64 * (t % 2):64 * (t % 2) + 64, :],
                            in_=c_bf[64 * par:64 * par + 64, :],
                        )
                    # fire down-conv pairs as soon as their f tiles are complete
                    if t % 2 == 1:
                        tt = t // 2
                        for hh in range(2):
                            nc.tensor.matmul(
                                o_ps[:],
                                w2h[hh][:, 128 * tt:128 * (tt + 1)],
                                f_sb[(hh, tt)][:],
                                start=(ndc == 0), stop=(ndc == 7),
                            )
                            ndc += 1

                # gate broadcast row -> [128, C], reusing the cc psum ring
                ybc_ps = psm.tile([128, C], F32, tag="cc", bufs=2, name="ybc_ps")
                nc.tensor.matmul(ybc_ps[:], ones_row, y2_sb[:],
                                 start=True, stop=True)
                nc.vector.tensor_copy(ybc_sb[:], ybc_ps[:])

                # merge + gate + stats
                ab = workp.tile([128, C], F32, tag="ab", bufs=1, name="ab")
                nc.vector.tensor_tensor(
                    out=ab[:], in0=oa_sb[:], in1=o_ps[:], op=ALU.add
                )
                sums = workp.tile([128, 2], F32, tag="sums", name="sums")
                nc.vector.scalar_tensor_tensor(
                    out=o_sb[:], in0=ab[:], scalar=1.0, in1=ybc_sb[:],
                    op0=ALU.mult, op1=ALU.mult, accum_out=sums[:, 0:1],
                )
                sqjunk = workp.tile([128, C], F32, tag="sqjunk", bufs=1, name="sqjunk")
                nc.scalar.activation(
                    sqjunk[:], o_sb[:], AF.Square, accum_out=sums[:, 1:2]
                )
                # cross-parity fold on PE: comb[p] = sums[p] + sums[p^64]
                comb_ps = psm.tile([128, 2], F32, tag="comb", bufs=1,
                                   name="comb_ps")
                nc.tensor.matmul(
                    comb_ps[:], blob[:, _FOLD:_FOLD + 128], sums[:],
                    start=True, stop=True,
                )
                stats2 = workp.tile([128, 2], F32, tag="stats2", name="stats2")
                nc.vector.tensor_scalar_mul(stats2[:], comb_ps[:], 1.0 / HW)
                m2 = workp.tile([128, 1], F32, tag="m2", name="m2")
                nc.vector.tensor_tensor(
                    out=m2[:], in0=stats2[:, 0:1], in1=stats2[:, 0:1], op=ALU.mult
                )
                var = workp.tile([128, 1], F32, tag="var", name="var")
                nc.vector.tensor_tensor(
                    out=var[:], in0=stats2[:, 1:2], in1=m2[:], op=ALU.subtract
                )
                std = workp.tile([128, 1], F32, tag="std", name="std")
                nc.scalar.activation(std[:], var[:], AF.Sqrt, bias=eps_v)
                norm2 = workp.tile([128, 2], F32, tag="norm2", name="norm2")
                nc.vector.reciprocal(norm2[:, 0:1], std[:])
                nc.vector.scalar_tensor_tensor(
                    out=norm2[:, 1:2], in0=stats2[:, 0:1], scalar=-1.0,
                    in1=norm2[:, 0:1], op0=ALU.mult, op1=ALU.mult,
                )
                outt = workp.tile([128, C], F32, tag="outt", bufs=1, name="outt")
                nc.scalar.activation(
                    outt[:], o_sb[:], AF.Prelu,
                    bias=norm2[:, 1:2], scale=norm2[:, 0:1], alpha=0.2,
                )
                nc.sync.dma_start(out=out_d[:, 0:C], in_=outt[0:64, :])
                nc.sync.dma_start(out=out_d[:, C:HW], in_=outt[64:128, :])

    return nc


def _split_drain_waits(nc, keep=1):
    """This walrus build allows at most 1 sync wait per instruction; hoist the
    extras onto preceding NoOps on the same engine."""
    n = 0
    for f in nc.m.functions:
        for bb in f.blocks:
            newlist = []
            for ins in bb.instructions:
                si = getattr(ins, "sync_info", None)
                if si is not None and si.on_wait and len(si.on_wait) > keep:
                    waits = list(si.on_wait)
                    for w in waits[:-keep]:
                        nop = mybir.InstNoOp(name=f"I-dw{n}", ins=[], outs=[])
                        n += 1
                        nop.engine = ins.engine
                        nop.sync_info = mybir.SyncInfo(on_wait=[w], on_update=[])
                        newlist.append(nop)
                    si.on_wait = waits[-keep:]
                newlist.append(ins)
            bb.instructions = newlist
    return n


_BUILT = None


def get_built():
    global _BUILT
    if _BUILT is None:
        nc = build_nc()
        _split_drain_waits(nc)
        _BUILT = nc
    return _BUILT


def kernel(x, se_w1, se_b1, se_w2, se_b2, down_w, _trace=False):
    in_maps = prep_inputs(x, se_w1, se_b1, se_w2, se_b2, down_w)
    nc = get_built()
    res = run_bass_kernel_spmd(nc, in_maps, list(range(NCORES)), trace=_trace)
    full = np.concatenate(
        [np.asarray(res.results[r]["out"], np.float32) for r in range(NCORES)], 0
    ).reshape(1, C, H, W)
    if _trace:
        return full, res
    return full
